# revision 2
# baseline (speedup 1.0000x reference)
"""MGU RNN (nn_Network_82394652607110) — Trainium2 Bass kernel.

Strategy
--------
Data-parallel over batch: 8 cores x 64 batch. Per core:

  Phase 1 (memory-bound): stream tx [64,1024,64] from HBM with casting
  gpsimd-DMA to bf16, xbar-transpose pairs of batches ([128 ts, (2b,64d)] ->
  [128 (b,d), 128 ts]), then project px = tx @ kernel on PE with a
  block-pair hi/lo-split bf16 kernel [[k,0],[0,k]] (two accumulating
  matmuls; px error ~2e-4). Outputs for 12 batches stack into one PSUM bank
  at partition offsets 20*jp, one ScalarE copy (+bias) moves each stack to
  SBUF, and SBUF->SBUF DMAs remap into the master layout.

  Master layout: partitions p = 5*g + u (g in 0..25 groups, u in 0..5),
  free = t (0..1024) in per-b_l-block tensors; batch slot b = 3*g + b_l
  (64 real + 11 pad slots).

  Phase 2 (the scan): the sequential recurrence
      v1 = sigmoid(p1 + h@Rf); v2 = tanh(p2 + (h*v1)@Rh)
      h' = (1-v1)*h + v1*v2
  is solved DEER-style: iterate [compute all gates for all t in parallel
  (block-diagonal matmuls over 25 groups + identity-matmul bias-add of
  p1/p2), then re-solve the now-linear recurrence h_t = w_t*h_{t-1} + m_t
  exactly with one tensor_tensor_scan per b_l block]. Converges to ~1e-5
  in 14 sweeps (verified against the exact sequential scan in numpy).

  Phase 3: logits = h_T @ fc_w + fc_b via one small matmul, softmax on
  device, DMA out [64, 4].
"""

import os
import numpy as np

import concourse.bass as bass
import concourse.bacc as bacc
import concourse.tile as tile
import concourse.mybir as mybir
from concourse.bass_utils import run_bass_kernel_spmd

dt = mybir.dt
AF = mybir.ActivationFunctionType
ALU = mybir.AluOpType

# Problem constants (hardcoded per harness contract)
U = 5
T = 1024
D = 64
B = 512
NCORES = 8
BC = B // NCORES  # 64 batch per core

# Master layout
G = 25            # partition groups
BL = 3            # batch slots per group
SLOTS = G * BL    # 75 (64 real + 11 pad)
P = G * U         # 125 partitions

NSWEEPS = int(os.environ.get("MGU_NSWEEPS", "14"))
# Gate matmuls run in fp16 (1 cyc/row on PE vs 4 for f32); verified final
# error ~9e-4 max / 1.6e-4 L2 in numpy simulation.
MM_DT = dt.float16
M_ON_GPSIMD = os.environ.get("MGU_M_GPSIMD", "1") == "1"

STACK = 6         # batches (3 pairs) stacked per psum bank at offsets 0/32/64


def build_program():
    nc = bacc.Bacc("TRN2", target_bir_lowering=False, debug=False)

    f32 = dt.float32
    bf16 = dt.float16  # fp16: 11-bit mantissa, 4x better px than bf16

    tx = nc.dram_tensor("tx", [BC, T, D], f32, kind="ExternalInput")
    # Pair-block kernel: [[k,0],[0,k]] hi/lo bf16 splits, [2D, 2*2U]
    khp_hi = nc.dram_tensor("khp_hi", [2 * D, 32], bf16, kind="ExternalInput")
    khp_lo = nc.dram_tensor("khp_lo", [2 * D, 32], bf16, kind="ExternalInput")
    bias96 = nc.dram_tensor("bias96", [96, 1], f32, kind="ExternalInput")
    bd_rf = nc.dram_tensor("bd_rf", [P, P], MM_DT, kind="ExternalInput")
    bd_rh = nc.dram_tensor("bd_rh", [P, P], MM_DT, kind="ExternalInput")
    ident = nc.dram_tensor("ident", [P, P], MM_DT, kind="ExternalInput")
    fcw6 = nc.dram_tensor("fcw6", [U + 1, 4], dt.float16, kind="ExternalInput")
    out = nc.dram_tensor("out", [BC, 4], f32, kind="ExternalOutput")

    with tile.TileContext(nc) as tc:
        with (
            tc.tile_pool(name="consts", bufs=1) as consts,
            tc.tile_pool(name="master", bufs=1) as master,
            tc.tile_pool(name="txbf", bufs=6) as txbf_pool,
            tc.tile_pool(name="xb", bufs=8) as xb_pool,
            tc.tile_pool(name="stg", bufs=3) as stg_pool,
            tc.tile_pool(name="ps1", bufs=2, space="PSUM") as ps1_pool,
            tc.tile_pool(name="ps2", bufs=3, space="PSUM") as ps2_pool,
            tc.tile_pool(name="gates", bufs=3) as gates_pool,
            tc.tile_pool(name="head", bufs=1) as head_pool,
        ):
            # ---- constants to SBUF ----
            kh_sb = consts.tile([2 * D, 32], bf16, tag="kh")
            kl_sb = consts.tile([2 * D, 32], bf16, tag="kl")
            b96_sb = consts.tile([96, 1], f32, tag="b96")
            bdrf_sb = consts.tile([P, P], MM_DT, tag="bdrf")
            bdrh_sb = consts.tile([P, P], MM_DT, tag="bdrh")
            id_sb = consts.tile([P, P], MM_DT, tag="ident")
            fcw_sb = consts.tile([U + 1, 4], dt.float16, tag="fcw")
            nc.sync.dma_start(kh_sb[:], khp_hi[:])
            nc.sync.dma_start(kl_sb[:], khp_lo[:])
            nc.sync.dma_start(b96_sb[:], bias96[:])
            nc.sync.dma_start(bdrf_sb[:], bd_rf[:])
            nc.sync.dma_start(bdrh_sb[:], bd_rh[:])
            nc.sync.dma_start(id_sb[:], ident[:])
            nc.sync.dma_start(fcw_sb[:], fcw6[:])

            # ---- persistent master-layout tensors ----
            # P1/P2 per b_l block: [125, 1024]; Hbuf per block: [125, 1025]
            # (col 0 is the t=0 zero state; scan writes cols 1..1025).
            P1 = [master.tile([P, T], dt.float16, tag=f"P1_{b}", name=f"P1_{b}")
                  for b in range(BL)]
            P2 = [master.tile([P, T], dt.float16, tag=f"P2_{b}", name=f"P2_{b}")
                  for b in range(BL)]
            Hb = [master.tile([P, T + 64], dt.float16, tag=f"Hb_{b}", name=f"Hb_{b}")
                  for b in range(BL)]
            for b in range(BL):
                nc.vector.memset(P1[b][:], 0.0)
                nc.vector.memset(P2[b][:], 0.0)
                nc.vector.memset(Hb[b][:], 0.0)

            # ---- Phase 1: px projection into master layout ----
            # b-groups of up to 6 batches (3 pairs at psum offsets 0/32/64);
            # t in halves of 512.
            groups = []
            b0 = 0
            while b0 < BC:
                groups.append((b0, min(STACK, BC - b0)))
                b0 += STACK
            for (gb0, gn) in groups:
                npairs = gn // 2
                tbfs = []
                for q in range(npairs):
                    # cast-load both batches of the pair, full T:
                    # [128 ts, 8 tb, 2 b, 64 d] bf16
                    tbf = txbf_pool.tile([128, 8, 2, D], bf16, tag="txbf")
                    for b01 in range(2):
                        b = gb0 + 2 * q + b01
                        nc.gpsimd.dma_start(
                            out=tbf[:, :, b01, :],
                            in_=tx[b].rearrange("(tb ts) d -> ts tb d", ts=128),
                        )
                    tbfs.append(tbf)
                for th in range(2):
                    ps = ps1_pool.tile([128, 512], dt.float32, tag="pxps")
                    for q in range(npairs):
                        for kk in range(4):
                            k = th * 4 + kk
                            xbt = xb_pool.tile([128, 128], bf16, tag="xb")
                            nc.sync.dma_start(
                                out=xbt[:], in_=tbfs[q][:, k, :, :],
                                transpose=True,
                            )
                            osl = ps[32 * q:32 * q + 32,
                                     128 * kk:128 * (kk + 1)]
                            nc.tensor.matmul(osl, lhsT=kh_sb[:], rhs=xbt[:],
                                             start=True, stop=False)
                            nc.tensor.matmul(osl, lhsT=kl_sb[:], rhs=xbt[:],
                                             start=False, stop=True)
                    # copy stack -> SBUF (+bias)
                    nrow = 32 * npairs
                    stg = stg_pool.tile([96, 512], dt.float16, tag="stg")
                    nc.scalar.activation(
                        stg[:nrow, :], ps[:nrow, :], AF.Identity,
                        bias=b96_sb[:nrow, :],
                    )
                    # remap into P1/P2: src row 32*(j//2) + 10*(j%2) + k
                    for j in range(gn):
                        b = gb0 + j
                        bl = b % 3
                        g = b // 3
                        row0 = 32 * (j // 2) + 10 * (j % 2)
                        for (dst, koff) in ((P1[bl], 0), (P2[bl], U)):
                            s_ap = stg[row0 + koff:row0 + koff + U, :]
                            d_ap = dst[5 * g:5 * g + 5,
                                       th * 512:(th + 1) * 512]
                            nc.sync.dma_start(out=d_ap, in_=s_ap)

            # ---- Phase 2: quasi-DEER sweeps ----
            for s in range(NSWEEPS):
                for bl in range(BL):
                    hprev = Hb[bl][:, 0:T]
                    pa = ps2_pool.tile([P, T], dt.float32, tag="ps2")
                    for c in range(2):
                        sl = slice(c * 512, (c + 1) * 512)
                        nc.tensor.matmul(pa[:, sl], lhsT=bdrf_sb[:],
                                         rhs=hprev[:, sl], start=True, stop=False)
                        nc.tensor.matmul(pa[:, sl], lhsT=id_sb[:],
                                         rhs=P1[bl][:, sl], start=False, stop=True)
                    v1 = gates_pool.tile([P, T], dt.float32, tag="v1")
                    nc.scalar.activation(v1[:], pa[:], AF.Sigmoid)
                    w = gates_pool.tile([P, T], dt.float32, tag="w")
                    nc.vector.tensor_scalar(w[:], v1[:], -1.0, 1.0,
                                            ALU.mult, ALU.add)
                    hv = gates_pool.tile([P, T], dt.float16, tag="hv")
                    nc.vector.tensor_tensor(hv[:], hprev, v1[:], ALU.mult)
                    pb = ps2_pool.tile([P, T], dt.float32, tag="ps2")
                    for c in range(2):
                        sl = slice(c * 512, (c + 1) * 512)
                        nc.tensor.matmul(pb[:, sl], lhsT=bdrh_sb[:],
                                         rhs=hv[:, sl], start=True, stop=False)
                        nc.tensor.matmul(pb[:, sl], lhsT=id_sb[:],
                                         rhs=P2[bl][:, sl], start=False, stop=True)
                    v2 = gates_pool.tile([P, T], dt.float32, tag="v2")
                    nc.scalar.activation(v2[:], pb[:], AF.Tanh)
                    m = gates_pool.tile([P, T], dt.float32, tag="m")
                    if M_ON_GPSIMD:
                        nc.gpsimd.tensor_tensor(m[:], v1[:], v2[:], ALU.mult)
                    else:
                        nc.vector.tensor_tensor(m[:], v1[:], v2[:], ALU.mult)
                    nc.vector.tensor_tensor_scan(
                        Hb[bl][:, 1:T + 1], w[:], m[:], 0.0, ALU.mult, ALU.add
                    )

            # ---- Phase 3: head ----
            hT = head_pool.tile([U + 1, SLOTS], dt.float16, tag="hT")
            nc.vector.memset(hT[:], 1.0)  # row U stays 1.0; rows 0..U overwritten below
            for bl in range(BL):
                for u in range(U):
                    s_ap = Hb[bl][:].rearrange("(g u) t -> g u t", u=U)[
                        :, u, T:T + 1]
                    d_ap = hT[u:u + 1, :].rearrange("p (g r) -> p g r", r=BL)[
                        :, :, bl]
                    nc.sync.dma_start(out=d_ap, in_=s_ap)
            pl = ps1_pool.tile([SLOTS, 4], dt.float32, tag="pxps")
            nc.tensor.matmul(pl[:], lhsT=hT[:], rhs=fcw_sb[:],
                             start=True, stop=True)
            nmax = head_pool.tile([SLOTS, 1], dt.float32, tag="nmax")
            nc.vector.tensor_reduce(nmax[:], pl[:], mybir.AxisListType.X,
                                    ALU.max, negate=True)
            ex = head_pool.tile([SLOTS, 4], dt.float32, tag="ex")
            nc.scalar.activation(ex[:], pl[:], AF.Exp, bias=nmax[:])
            sm = head_pool.tile([SLOTS, 1], dt.float32, tag="sm")
            nc.vector.tensor_reduce(sm[:], ex[:], mybir.AxisListType.X, ALU.add)
            ri = head_pool.tile([SLOTS, 1], dt.float32, tag="ri")
            nc.vector.reciprocal(ri[:], sm[:])
            op = head_pool.tile([SLOTS, 4], dt.float32, tag="op")
            nc.vector.tensor_scalar(op[:], ex[:], ri[:], None, ALU.mult)
            nc.sync.dma_start(out=out[:], in_=op[0:BC, :])

    nc.compile()
    return nc


def _prep_host_inputs(kernel, rec_kernel, bias, fc_w, fc_b):
    f32 = np.float32
    k = np.asarray(kernel, f32)          # [64, 10]
    k_hi = k.astype(np.float16)
    k_lo = (k - k_hi.astype(f32)).astype(np.float16)

    def pairblock(kk):
        z = np.zeros((2 * D, 32), kk.dtype)
        z[:D, :2 * U] = kk
        z[D:, 2 * U:4 * U] = kk
        return z

    b96 = np.zeros((96, 1), f32)
    bias_f = np.asarray(bias, f32)
    for q in range(3):
        for b01 in range(2):
            b96[32 * q + 10 * b01:32 * q + 10 * b01 + 10, 0] = bias_f

    rk = np.asarray(rec_kernel, f32)
    bd_rf = np.zeros((P, P), np.float16)
    bd_rh = np.zeros((P, P), np.float16)
    for g in range(G):
        bd_rf[5 * g:5 * g + 5, 5 * g:5 * g + 5] = rk[:, :U]
        bd_rh[5 * g:5 * g + 5, 5 * g:5 * g + 5] = rk[:, U:]
    ident = np.eye(P, dtype=np.float16)

    fcw6 = np.concatenate([np.asarray(fc_w, f32),
                           np.asarray(fc_b, f32)[None, :]],
                          axis=0).astype(np.float16)
    return dict(khp_hi=pairblock(k_hi), khp_lo=pairblock(k_lo), bias96=b96,
                bd_rf=bd_rf, bd_rh=bd_rh, ident=ident, fcw6=fcw6)


_CACHE = {}


def kernel(tx, kernel, rec_kernel, bias, fc_w, fc_b, _want_time=False):
    tx = np.ascontiguousarray(np.asarray(tx, np.float32))
    host = _prep_host_inputs(kernel, rec_kernel, bias, fc_w, fc_b)

    if "nc" not in _CACHE:
        _CACHE["nc"] = build_program()
    nc = _CACHE["nc"]

    in_maps = []
    for c in range(NCORES):
        m = {"tx": np.ascontiguousarray(tx[c * BC:(c + 1) * BC])}
        m.update(host)
        in_maps.append(m)

    try:
        res = run_bass_kernel_spmd(
            nc, in_maps, core_ids=list(range(NCORES)), trace=_want_time
        )
    except ModuleNotFoundError:
        # axon NTFF profile hook unavailable: run without tracing
        res = run_bass_kernel_spmd(
            nc, in_maps, core_ids=list(range(NCORES)), trace=False
        )
    outs = [res.results[c]["out"] for c in range(NCORES)]
    full = np.concatenate(outs, axis=0)
    if _want_time:
        _CACHE["res"] = res
        return full, res.exec_time_ns
    return full

